# revision 44
# baseline (speedup 1.0000x reference)
"""Trainium2 Bass kernel for nn_CT_37821482009425 (snntorch Leaky LIF scan).

Reference semantics (bitwise-matched):
    T = clip(t, 1, 5); x = roll(inp, roll_amount, axis=2)
    per step: reset = (mem > T); mem = 0.95*mem + x_t - reset*T; spk = (mem > T)
Output: spikes (1024, 1, 224, 224) float32 in {0, 1}.

Distribution: pure data parallelism — batch 1024 -> 8 cores x 128 (the SBUF
partition dim). Host prep per core: apply the roll and transpose to
time-major so each timestep's H=224 vector is contiguous per partition.

Per-core compute, ONE fused custom-DVE instruction per time step:
    mem_t = (mem_{t-1}*beta + x_t) - T*(mem_{t-1} > T)
The uOp chain rounds each ALU stage to fp32 in the same order as the
reference ((beta*mem) + x, then the exact {0,T} subtraction), so mem is
bit-exact. The op is registered in concourse's per-NEFF custom-DVE table;
its InstCustomDveAnt must be lowered to encoded ISA bytes via
codegen_inst_isa_one before serialization (raw Bass skips that pass, and
walrus rejects the empty-instr form with "ISA wrong length").

Spikes are recovered from mem on the Activation engine as sign(mem - T)
cast to int8 (exact: sign of an fp32 subtract has the sign of the true
difference), cutting output DMA to 1 byte/element; the host maps
v > 0 -> 1.0f. For 12 evenly-spread slices, DVE additionally packs sign
pairs into one byte (odd*4 + even, one strided int8 STT per slice),
halving those slices' output bytes: that balances the DVE chain against
the serialized DMA device, which this kernel saturates end-to-end
(~80% input stream at the model's 360 GB/s, ~20% packed spike output).
The sync engine streams input in 8-step slices with a 12-slice prefetch
ring; mem/sign rings are sized so nothing ever waits on the out-DMAs
that queue behind the initial input burst on the FIFO DMA device.
"""

import numpy as np
import concourse.bass as bass
import concourse.mybir as mybir
import concourse.bass_isa as bass_isa
from concourse.bass_utils import run_bass_kernel_spmd

BETA = 0.95
B, CH = 1024, 224
N_CORES = 8
PB = B // N_CORES  # 128 batches per core = partition dim
H = CH  # per-step vector length (contiguous, time-major)
W = CH  # time steps
SUB = 8  # steps per slice (DMA + pipeline granularity)
N_SUB = W // SUB  # 28 slices
RING_Z = 12  # input-slice ring depth (in-DMA prefetch window)
RING_M = 4   # mem-tile ring depth (consumed by ACT within ~1 slice)
RING_S = 12  # sign-tile ring depth (deep: out-DMAs queue behind the initial
RING_P = 12  # in-DMA burst on the FIFO DMA device, so sb/pb slots must not
#              be needed again until that burst drains)
# Output packing: for PACKED slices DVE packs the int8 sign pairs
# (odd*4 + even) so their out-DMA moves half the bytes. The count balances
# the DVE chain (+~1.0us per packed slice) against the serialized DMA
# device (-~0.32us per packed slice): the DVE-side critical path
# (fill + 65.8 + 0.994N + tail) crosses the DMA-side one
# (lead + 89.2 - 0.319N) near N=12 (swept empirically; Pool cannot help:
# its ISA rejects int8 adds). Spread evenly; the last slice stays
# unpacked to keep the drain tail short.
NPACK = 12
PACKED = tuple(
    s for s in range(N_SUB - 1)
    if (s * NPACK) // (N_SUB - 1) != ((s + 1) * NPACK) // (N_SUB - 1)
)
PIDX = {s: i for i, s in enumerate(PACKED)}
_Alu = mybir.AluOpType

_cache = {}
_lif_op = None


def _get_lif_op():
    """Register the fused LIF-step op in concourse's custom-DVE table.

    body: out = (Src0*C0 + Src1) - C1*(Src0 > C1)
          (Src0=mem_{t-1}, Src1=x_t, C0=beta, C1=T)
    """
    global _lif_op
    if _lif_op is not None:
        return _lif_op
    import concourse.dve_ops as dve_ops
    from concourse.dve_ops import DveOp
    from concourse.dve_spec import Spec, Src0, Src1, C0, C1, lower, _has_src1
    from concourse.dve_uop import DveOpSpec

    name = "LIF_STEP_ANT"
    for op in dve_ops.OPS:
        if op.name == name:
            _lif_op = op
            return op

    body = (Src0 * C0 + Src1) - C1 * (Src0 > C1)

    def ref(in0, in1, s0, s1, imm2):
        in0 = np.asarray(in0, np.float32)
        in1 = np.asarray(in1, np.float32)
        b = np.float32(s0) if not isinstance(s0, np.ndarray) else s0.astype(np.float32)
        t = np.float32(s1) if not isinstance(s1, np.ndarray) else s1.astype(np.float32)
        p = (in0 > t).astype(np.float32)
        return ((in0 * b + in1) - t * p).astype(np.float32)

    spec = Spec(body=body, reference=ref)
    row = max(dve_ops._SUB_OPCODE_FOR_NAME.values()) + 1
    assert row < 0x20
    op = DveOp(name, spec, subdim=False, uops_sha={})
    # sha is self-computed: pins the lowering within this process, the same
    # check dve_table_for_ops performs at compile time.
    for ver in ("v3", "v4"):
        s = DveOpSpec(
            name=name, opcode=row, uops=lower(spec, ver=ver), rd1_en=_has_src1(spec)
        )
        op.uops_sha[ver] = s.sha(ver)
    dve_ops._SUB_OPCODE_FOR_NAME[name] = row
    dve_ops.OPS.append(op)
    dve_ops.CUSTOM_DVE_SPECS[name] = spec
    _lif_op = op
    return op


def _lower_custom_dve(nc):
    """Encode InstCustomDveAnt instructions to ISA bytes in place.

    Raw Bass never runs Bacc's codegen_inst_isa_subclasses pass (and the
    blanket pass would also rewrite DMA copies, which this walrus rejects),
    so lower just the custom-DVE instructions.
    """
    for f in nc.m.functions:
        for bb in f.blocks:
            for i, inst in enumerate(bb.instructions):
                if isinstance(inst, bass_isa.InstCustomDveAnt):
                    lowered = mybir.codegen_inst_isa_one(inst, nc._state, nc.isa)
                    if isinstance(lowered, list):
                        assert len(lowered) == 1
                        lowered = lowered[0]
                    bb.instructions[i] = lowered


def _build(T: float):
    lif = _get_lif_op()
    nc = bass.Bass(trn_type="TRN2")
    x_d = nc.dram_tensor("x", [PB, W * H], mybir.dt.float32, kind="ExternalInput")
    r_d = nc.dram_tensor("r", [PB, W * H], mybir.dt.int8, kind="ExternalOutput")

    SL = SUB * H  # elements per slice (per partition)

    with (
        nc.sbuf_tensor("xb", [PB, RING_Z * SL], mybir.dt.float32) as xb,
        nc.sbuf_tensor("mb", [PB, RING_M * SL], mybir.dt.float32) as mb,
        nc.sbuf_tensor("sb", [PB, RING_S * SL], mybir.dt.int8) as sb,
        nc.sbuf_tensor("pb", [PB, RING_P * (SL // 2)], mybir.dt.int8) as pb,
        nc.sbuf_tensor("z0", [PB, H], mybir.dt.float32) as z0,
        nc.sbuf_tensor("nt", [PB, 1], mybir.dt.float32) as nt,
        nc.semaphore() as in_sem,
        nc.semaphore() as v_sem,
        nc.semaphore() as p_sem,
        nc.semaphore() as act_sem,
        nc.semaphore() as out_sem,
        nc.Block() as block,
    ):

        @block.sync
        def _(sync):
            for s in range(N_SUB):
                if s >= RING_Z:
                    # x slot free once DVE consumed slice s-RING_Z
                    sync.wait_ge(v_sem, s - RING_Z + 1)
                sync.dma_start(
                    xb[:, (s % RING_Z) * SL : (s % RING_Z + 1) * SL],
                    x_d[:, s * SL : (s + 1) * SL],
                ).then_inc(in_sem, 16)

        @block.vector
        def _(vector):
            nc.vector.memset(z0[:], 0.0)
            nc.vector.memset(nt[:], -T)
            for s in range(N_SUB):
                vector.wait_ge(in_sem, 16 * (s + 1))
                if s >= RING_M:
                    # mem slot free once ACT signed slice s-RING_M
                    vector.wait_ge(act_sem, s - RING_M + 1)
                for j in range(SUB):
                    t = s * SUB + j
                    if t == 0:
                        src0 = z0[:]
                    else:
                        ps, pj = divmod(t - 1, SUB)
                        src0 = mb[
                            :, (ps % RING_M) * SL + pj * H : (ps % RING_M) * SL + (pj + 1) * H
                        ]
                    ins = nc.vector._custom_dve(
                        lif,
                        out=mb[:, (s % RING_M) * SL + j * H : (s % RING_M) * SL + (j + 1) * H],
                        in0=src0,
                        in1=xb[:, (s % RING_Z) * SL + j * H : (s % RING_Z) * SL + (j + 1) * H],
                        s0=BETA,
                        s1=T,
                    )
                    if j == SUB - 1:
                        ins.then_inc(v_sem, 1)
                # pack slice s-1 (one slice of lag hides ACT's sign latency)
                k = s - 1
                if k in PIDX:
                    kb = (k % RING_S) * SL
                    if k >= RING_P:
                        # pb slot free once out-DMA of slice k-RING_P completed
                        vector.wait_ge(out_sem, 16 * (k - RING_P + 1))
                    vector.wait_ge(act_sem, k + 1)
                    nc.vector.scalar_tensor_tensor(
                        pb[:, (k % RING_P) * (SL // 2) : (k % RING_P + 1) * (SL // 2)],
                        sb[:, kb + 1 : kb + SL : 2],
                        4.0,
                        sb[:, kb : kb + SL : 2],
                        _Alu.mult,
                        _Alu.add,
                    ).then_inc(p_sem, 1)

        @block.scalar
        def _(scalar):
            def emit_dma(k):
                # out-DMA for slice k, issued one slice late so its producer
                # (DVE's pack, or this engine's own sign) has already landed
                # by the time the SEQ reaches the wait.
                if k in PIDX:
                    scalar.wait_ge(p_sem, PIDX[k] + 1)
                    scalar.dma_start(
                        r_d[:, k * SL : k * SL + SL // 2],
                        pb[:, (k % RING_P) * (SL // 2) : (k % RING_P + 1) * (SL // 2)],
                    ).then_inc(out_sem, 16)
                else:
                    scalar.wait_ge(act_sem, k + 1)
                    scalar.dma_start(
                        r_d[:, k * SL : (k + 1) * SL],
                        sb[:, (k % RING_S) * SL : (k % RING_S + 1) * SL],
                    ).then_inc(out_sem, 16)

            for s in range(N_SUB):
                if s >= RING_S:
                    # sb slot (slice s-RING_S) free once its consumer is done:
                    # DVE's pack for packed slices, the out-DMA otherwise
                    k = s - RING_S
                    if k in PIDX:
                        scalar.wait_ge(p_sem, PIDX[k] + 1)
                    else:
                        scalar.wait_ge(out_sem, 16 * (k + 1))
                base = (s % RING_S) * SL
                scalar.wait_ge(v_sem, s + 1)
                nc.scalar.sign(
                    sb[:, base : base + SL],
                    mb[:, (s % RING_M) * SL : (s % RING_M + 1) * SL],
                    bias=nt[:],
                ).then_inc(act_sem, 1)
                if s >= 1:
                    emit_dma(s - 1)
            emit_dma(N_SUB - 1)

    _lower_custom_dve(nc)
    return nc


def kernel(inp: np.ndarray, t: np.ndarray, roll_amount) -> np.ndarray:
    T = float(
        np.clip(np.float32(np.asarray(t).reshape(-1)[0]), np.float32(1.0),
                np.float32(5.0))
    )
    roll = int(np.asarray(roll_amount)) % W

    key = (T,)
    if key not in _cache:
        _cache[key] = _build(T)
    nc = _cache[key]

    inp = np.asarray(inp, dtype=np.float32).reshape(B, CH, CH)
    in_maps = []
    for c in range(N_CORES):
        shard = inp[c * PB : (c + 1) * PB]  # (128, H, W)
        shard = np.roll(shard, roll, axis=2)
        # time-major: (128, W, H) contiguous
        x_tm = np.ascontiguousarray(shard.transpose(0, 2, 1)).reshape(PB, W * H)
        in_maps.append({"x": x_tm})

    res = run_bass_kernel_spmd(nc, in_maps, core_ids=list(range(N_CORES)))

    SL = SUB * H
    out = np.empty((B, 1, CH, CH), dtype=np.float32)
    for c in range(N_CORES):
        r = res.results[c]["r"]  # (PB, W*H) int8; packed slices use half a slot
        spk = np.empty((PB, W * H), dtype=bool)
        for s in range(N_SUB):
            if s in PIDX:
                v = r[:, s * SL : s * SL + SL // 2].astype(np.int32)
                odd = np.rint(v / 4.0).astype(np.int32)  # in {-1, 0, 1}
                even = v - 4 * odd
                sl = np.empty((PB, SL), dtype=bool)
                sl[:, 0::2] = even > 0
                sl[:, 1::2] = odd > 0
                spk[:, s * SL : (s + 1) * SL] = sl
            else:
                spk[:, s * SL : (s + 1) * SL] = r[:, s * SL : (s + 1) * SL] > 0
        spk = spk.reshape(PB, W, H)  # (b, w, h)
        out[c * PB : (c + 1) * PB, 0] = spk.transpose(0, 2, 1).astype(np.float32)
    return out


# revision 49
# speedup vs baseline: 1.0034x; 1.0034x over previous
"""Trainium2 Bass kernel for nn_CT_37821482009425 (snntorch Leaky LIF scan).

Reference semantics (bitwise-matched):
    T = clip(t, 1, 5); x = roll(inp, roll_amount, axis=2)
    per step: reset = (mem > T); mem = 0.95*mem + x_t - reset*T; spk = (mem > T)
Output: spikes (1024, 1, 224, 224) float32 in {0, 1}.

Distribution: pure data parallelism — batch 1024 -> 8 cores x 128 (the SBUF
partition dim). Host prep per core: apply the roll and transpose to
time-major so each timestep's H=224 vector is contiguous per partition.

Per-core compute, ONE fused custom-DVE instruction per time step:
    mem_t = (mem_{t-1}*beta + x_t) - T*(mem_{t-1} > T)
The uOp chain rounds each ALU stage to fp32 in the same order as the
reference ((beta*mem) + x, then the exact {0,T} subtraction), so mem is
bit-exact. The op is registered in concourse's per-NEFF custom-DVE table;
its InstCustomDveAnt must be lowered to encoded ISA bytes via
codegen_inst_isa_one before serialization (raw Bass skips that pass, and
walrus rejects the empty-instr form with "ISA wrong length").

Spikes are recovered from mem on the Activation engine as sign(mem - T)
cast to int8 (exact: sign of an fp32 subtract has the sign of the true
difference), cutting output DMA to 1 byte/element; the host maps
v > 0 -> 1.0f. For 12 evenly-spread slices, DVE additionally packs sign
pairs into one byte (odd*4 + even, one strided int8 STT per slice),
halving those slices' output bytes: that balances the DVE chain against
the serialized DMA device, which this kernel saturates end-to-end
(~80% input stream at the model's 360 GB/s, ~20% packed spike output).
The sync engine streams input in 8-step slices with a 12-slice prefetch
ring; mem/sign rings are sized so nothing ever waits on the out-DMAs
that queue behind the initial input burst on the FIFO DMA device.
"""

import numpy as np
import concourse.bass as bass
import concourse.mybir as mybir
import concourse.bass_isa as bass_isa
from concourse.bass_utils import run_bass_kernel_spmd

BETA = 0.95
B, CH = 1024, 224
N_CORES = 8
PB = B // N_CORES  # 128 batches per core = partition dim
H = CH  # per-step vector length (contiguous, time-major)
W = CH  # time steps
SUB = 8  # steps per slice (DMA + pipeline granularity)
N_SUB = W // SUB  # 28 slices
RING_Z = 12  # input-slice ring depth (in-DMA prefetch window)
RING_M = 4   # mem-tile ring depth (consumed by ACT within ~1 slice)
RING_S = 12  # sign-tile ring depth (deep: out-DMAs queue behind the initial
RING_P = 12  # in-DMA burst on the FIFO DMA device, so sb/pb slots must not
#              be needed again until that burst drains)
# Output packing: for PACKED slices DVE packs the int8 sign pairs
# (odd*4 + even) so their out-DMA moves half the bytes. The count balances
# the DVE chain (+~1.0us per packed slice) against the serialized DMA
# device (-~0.32us per packed slice): the DVE-side critical path
# (fill + 65.8 + 0.994N + tail) crosses the DMA-side one
# (lead + 89.2 - 0.319N) near N=12 (swept empirically; Pool cannot help:
# its ISA rejects int8 adds). Spread evenly; the last slice stays
# unpacked to keep the drain tail short.
NPACK = 13
PACKED = tuple(
    s for s in range(N_SUB - 2)
    if (s * NPACK) // (N_SUB - 2) != ((s + 1) * NPACK) // (N_SUB - 2)
)
PIDX = {s: i for i, s in enumerate(PACKED)}
_Alu = mybir.AluOpType

_cache = {}
_lif_op = None


def _get_lif_op():
    """Register the fused LIF-step op in concourse's custom-DVE table.

    body: out = (Src0*C0 + Src1) - C1*(Src0 > C1)
          (Src0=mem_{t-1}, Src1=x_t, C0=beta, C1=T)
    """
    global _lif_op
    if _lif_op is not None:
        return _lif_op
    import concourse.dve_ops as dve_ops
    from concourse.dve_ops import DveOp
    from concourse.dve_spec import Spec, Src0, Src1, C0, C1, lower, _has_src1
    from concourse.dve_uop import DveOpSpec

    name = "LIF_STEP_ANT"
    for op in dve_ops.OPS:
        if op.name == name:
            _lif_op = op
            return op

    body = (Src0 * C0 + Src1) - C1 * (Src0 > C1)

    def ref(in0, in1, s0, s1, imm2):
        in0 = np.asarray(in0, np.float32)
        in1 = np.asarray(in1, np.float32)
        b = np.float32(s0) if not isinstance(s0, np.ndarray) else s0.astype(np.float32)
        t = np.float32(s1) if not isinstance(s1, np.ndarray) else s1.astype(np.float32)
        p = (in0 > t).astype(np.float32)
        return ((in0 * b + in1) - t * p).astype(np.float32)

    spec = Spec(body=body, reference=ref)
    row = max(dve_ops._SUB_OPCODE_FOR_NAME.values()) + 1
    assert row < 0x20
    op = DveOp(name, spec, subdim=False, uops_sha={})
    # sha is self-computed: pins the lowering within this process, the same
    # check dve_table_for_ops performs at compile time.
    for ver in ("v3", "v4"):
        s = DveOpSpec(
            name=name, opcode=row, uops=lower(spec, ver=ver), rd1_en=_has_src1(spec)
        )
        op.uops_sha[ver] = s.sha(ver)
    dve_ops._SUB_OPCODE_FOR_NAME[name] = row
    dve_ops.OPS.append(op)
    dve_ops.CUSTOM_DVE_SPECS[name] = spec
    _lif_op = op
    return op


def _lower_custom_dve(nc):
    """Encode InstCustomDveAnt instructions to ISA bytes in place.

    Raw Bass never runs Bacc's codegen_inst_isa_subclasses pass (and the
    blanket pass would also rewrite DMA copies, which this walrus rejects),
    so lower just the custom-DVE instructions.
    """
    for f in nc.m.functions:
        for bb in f.blocks:
            for i, inst in enumerate(bb.instructions):
                if isinstance(inst, bass_isa.InstCustomDveAnt):
                    lowered = mybir.codegen_inst_isa_one(inst, nc._state, nc.isa)
                    if isinstance(lowered, list):
                        assert len(lowered) == 1
                        lowered = lowered[0]
                    bb.instructions[i] = lowered


def _build(T: float):
    lif = _get_lif_op()
    nc = bass.Bass(trn_type="TRN2")
    x_d = nc.dram_tensor("x", [PB, W * H], mybir.dt.float32, kind="ExternalInput")
    r_d = nc.dram_tensor("r", [PB, W * H], mybir.dt.int8, kind="ExternalOutput")

    SL = SUB * H  # elements per slice (per partition)

    with (
        nc.sbuf_tensor("xb", [PB, RING_Z * SL], mybir.dt.float32) as xb,
        nc.sbuf_tensor("mb", [PB, RING_M * SL], mybir.dt.float32) as mb,
        nc.sbuf_tensor("sb", [PB, RING_S * SL], mybir.dt.int8) as sb,
        nc.sbuf_tensor("pb", [PB, RING_P * (SL // 2)], mybir.dt.int8) as pb,
        nc.sbuf_tensor("z0", [PB, H], mybir.dt.float32) as z0,
        nc.sbuf_tensor("nt", [PB, 1], mybir.dt.float32) as nt,
        nc.semaphore() as in_sem,
        nc.semaphore() as v_sem,
        nc.semaphore() as p_sem,
        nc.semaphore() as act_sem,
        nc.semaphore() as out_sem,
        nc.Block() as block,
    ):

        @block.sync
        def _(sync):
            for s in range(N_SUB):
                if s >= RING_Z:
                    # x slot free once DVE consumed slice s-RING_Z
                    sync.wait_ge(v_sem, s - RING_Z + 1)
                sync.dma_start(
                    xb[:, (s % RING_Z) * SL : (s % RING_Z + 1) * SL],
                    x_d[:, s * SL : (s + 1) * SL],
                ).then_inc(in_sem, 16)
            # drain out-DMAs: SP is idle once the input stream is issued, and
            # its DMA-issue path (SEQ+HWDGE+DGE) overlaps ACT's sign pipeline
            # instead of queueing behind it.
            sd = N_SUB - 1
            HS2 = SL // 2
            sync.wait_ge(act_sem, N_SUB - 1)
            sync.dma_start(
                r_d[:, (sd - 1) * SL : sd * SL],
                sb[:, ((sd - 1) % RING_S) * SL : ((sd - 1) % RING_S + 1) * SL],
            ).then_inc(out_sem, 16)
            for h in range(2):
                sync.wait_ge(act_sem, N_SUB + h)
                sync.dma_start(
                    r_d[:, sd * SL + h * HS2 : sd * SL + (h + 1) * HS2],
                    sb[:, (sd % RING_S) * SL + h * HS2 : (sd % RING_S) * SL + (h + 1) * HS2],
                ).then_inc(out_sem, 16)

        @block.vector
        def _(vector):
            nc.vector.memset(z0[:], 0.0)
            nc.vector.memset(nt[:], -T)
            for s in range(N_SUB):
                vector.wait_ge(in_sem, 16 * (s + 1))
                if s >= RING_M:
                    # mem slot free once ACT signed slice s-RING_M
                    vector.wait_ge(act_sem, s - RING_M + 1)
                for j in range(SUB):
                    t = s * SUB + j
                    if t == 0:
                        src0 = z0[:]
                    else:
                        ps, pj = divmod(t - 1, SUB)
                        src0 = mb[
                            :, (ps % RING_M) * SL + pj * H : (ps % RING_M) * SL + (pj + 1) * H
                        ]
                    ins = nc.vector._custom_dve(
                        lif,
                        out=mb[:, (s % RING_M) * SL + j * H : (s % RING_M) * SL + (j + 1) * H],
                        in0=src0,
                        in1=xb[:, (s % RING_Z) * SL + j * H : (s % RING_Z) * SL + (j + 1) * H],
                        s0=BETA,
                        s1=T,
                    )
                    if s == N_SUB - 1:
                        # half-slice completion so the drain's sign/out-DMA
                        # can start on the first half early
                        if j in (SUB // 2 - 1, SUB - 1):
                            ins.then_inc(v_sem, 1)
                    elif j == SUB - 1:
                        ins.then_inc(v_sem, 1)
                # pack slice s-1 (one slice of lag hides ACT's sign latency)
                k = s - 1
                if k in PIDX:
                    kb = (k % RING_S) * SL
                    if k >= RING_P:
                        # pb slot free once out-DMA of slice k-RING_P completed
                        vector.wait_ge(out_sem, 16 * (k - RING_P + 1))
                    vector.wait_ge(act_sem, k + 1)
                    nc.vector.scalar_tensor_tensor(
                        pb[:, (k % RING_P) * (SL // 2) : (k % RING_P + 1) * (SL // 2)],
                        sb[:, kb + 1 : kb + SL : 2],
                        4.0,
                        sb[:, kb : kb + SL : 2],
                        _Alu.mult,
                        _Alu.add,
                    ).then_inc(p_sem, 1)

        @block.scalar
        def _(scalar):
            def emit_dma(k):
                # out-DMA for slice k, issued one slice late so its producer
                # (DVE's pack, or this engine's own sign) has already landed
                # by the time the SEQ reaches the wait.
                if k in PIDX:
                    scalar.wait_ge(p_sem, PIDX[k] + 1)
                    scalar.dma_start(
                        r_d[:, k * SL : k * SL + SL // 2],
                        pb[:, (k % RING_P) * (SL // 2) : (k % RING_P + 1) * (SL // 2)],
                    ).then_inc(out_sem, 16)
                else:
                    scalar.wait_ge(act_sem, k + 1)
                    scalar.dma_start(
                        r_d[:, k * SL : (k + 1) * SL],
                        sb[:, (k % RING_S) * SL : (k % RING_S + 1) * SL],
                    ).then_inc(out_sem, 16)

            for s in range(N_SUB - 1):
                if s >= RING_S:
                    # sb slot (slice s-RING_S) free once its consumer is done:
                    # DVE's pack for packed slices, the out-DMA otherwise
                    k = s - RING_S
                    if k in PIDX:
                        scalar.wait_ge(p_sem, PIDX[k] + 1)
                    else:
                        scalar.wait_ge(out_sem, 16 * (k + 1))
                base = (s % RING_S) * SL
                scalar.wait_ge(v_sem, s + 1)
                nc.scalar.sign(
                    sb[:, base : base + SL],
                    mb[:, (s % RING_M) * SL : (s % RING_M + 1) * SL],
                    bias=nt[:],
                ).then_inc(act_sem, 1)
                if s >= 1:
                    emit_dma(s - 1)
            # drain: slice 27 in two 4-step halves. v_sem counts 27 whole
            # slices then one per half (28, 29); act_sem likewise. Slice 26
            # is unpacked so its DMA (ready as soon as sign 26 landed) issues
            # before the halves; the final half-DMA needs no semaphore update
            # (nothing consumes it), saving the trailing propagation delay.
            s = N_SUB - 1
            k = s - RING_S
            if k in PIDX:
                scalar.wait_ge(p_sem, PIDX[k] + 1)
            else:
                scalar.wait_ge(out_sem, 16 * (k + 1))
            base = (s % RING_S) * SL
            mbase = (s % RING_M) * SL
            HS = SL // 2
            for h in range(2):
                scalar.wait_ge(v_sem, s + h + 1)
                nc.scalar.sign(
                    sb[:, base + h * HS : base + (h + 1) * HS],
                    mb[:, mbase + h * HS : mbase + (h + 1) * HS],
                    bias=nt[:],
                ).then_inc(act_sem, 1)

    _lower_custom_dve(nc)
    return nc


def kernel(inp: np.ndarray, t: np.ndarray, roll_amount) -> np.ndarray:
    T = float(
        np.clip(np.float32(np.asarray(t).reshape(-1)[0]), np.float32(1.0),
                np.float32(5.0))
    )
    roll = int(np.asarray(roll_amount)) % W

    key = (T,)
    if key not in _cache:
        _cache[key] = _build(T)
    nc = _cache[key]

    inp = np.asarray(inp, dtype=np.float32).reshape(B, CH, CH)
    in_maps = []
    for c in range(N_CORES):
        shard = inp[c * PB : (c + 1) * PB]  # (128, H, W)
        shard = np.roll(shard, roll, axis=2)
        # time-major: (128, W, H) contiguous
        x_tm = np.ascontiguousarray(shard.transpose(0, 2, 1)).reshape(PB, W * H)
        in_maps.append({"x": x_tm})

    res = run_bass_kernel_spmd(nc, in_maps, core_ids=list(range(N_CORES)))

    SL = SUB * H
    out = np.empty((B, 1, CH, CH), dtype=np.float32)
    for c in range(N_CORES):
        r = res.results[c]["r"]  # (PB, W*H) int8; packed slices use half a slot
        spk = np.empty((PB, W * H), dtype=bool)
        for s in range(N_SUB):
            if s in PIDX:
                v = r[:, s * SL : s * SL + SL // 2].astype(np.int32)
                odd = np.rint(v / 4.0).astype(np.int32)  # in {-1, 0, 1}
                even = v - 4 * odd
                sl = np.empty((PB, SL), dtype=bool)
                sl[:, 0::2] = even > 0
                sl[:, 1::2] = odd > 0
                spk[:, s * SL : (s + 1) * SL] = sl
            else:
                spk[:, s * SL : (s + 1) * SL] = r[:, s * SL : (s + 1) * SL] > 0
        spk = spk.reshape(PB, W, H)  # (b, w, h)
        out[c * PB : (c + 1) * PB, 0] = spk.transpose(0, 2, 1).astype(np.float32)
    return out


# revision 52
# speedup vs baseline: 1.0088x; 1.0054x over previous
"""Trainium2 Bass kernel for nn_CT_37821482009425 (snntorch Leaky LIF scan).

Reference semantics (bitwise-matched):
    T = clip(t, 1, 5); x = roll(inp, roll_amount, axis=2)
    per step: reset = (mem > T); mem = 0.95*mem + x_t - reset*T; spk = (mem > T)
Output: spikes (1024, 1, 224, 224) float32 in {0, 1}.

Distribution: pure data parallelism — batch 1024 -> 8 cores x 128 (the SBUF
partition dim). Host prep per core: apply the roll and transpose to
time-major so each timestep's H=224 vector is contiguous per partition.

Per-core compute, ONE fused custom-DVE instruction per time step:
    mem_t = (mem_{t-1}*beta + x_t) - T*(mem_{t-1} > T)
The uOp chain rounds each ALU stage to fp32 in the same order as the
reference ((beta*mem) + x, then the exact {0,T} subtraction), so mem is
bit-exact. The op is registered in concourse's per-NEFF custom-DVE table;
its InstCustomDveAnt must be lowered to encoded ISA bytes via
codegen_inst_isa_one before serialization (raw Bass skips that pass, and
walrus rejects the empty-instr form with "ISA wrong length").

Spikes are recovered from mem on the Activation engine as sign(mem - T)
cast to int8 (exact: sign of an fp32 subtract has the sign of the true
difference), cutting output DMA to 1 byte/element; the host maps
v > 0 -> 1.0f. For 12 evenly-spread slices, DVE additionally packs sign
pairs into one byte (odd*4 + even, one strided int8 STT per slice),
halving those slices' output bytes: that balances the DVE chain against
the serialized DMA device, which this kernel saturates end-to-end
(~80% input stream at the model's 360 GB/s, ~20% packed spike output).
The sync engine streams input in 8-step slices with a 12-slice prefetch
ring; mem/sign rings are sized so nothing ever waits on the out-DMAs
that queue behind the initial input burst on the FIFO DMA device.
"""

import numpy as np
import concourse.bass as bass
import concourse.mybir as mybir
import concourse.bass_isa as bass_isa
from concourse.bass_utils import run_bass_kernel_spmd

BETA = 0.95
B, CH = 1024, 224
N_CORES = 8
PB = B // N_CORES  # 128 batches per core = partition dim
H = CH  # per-step vector length (contiguous, time-major)
W = CH  # time steps
SUB = 8  # steps per slice (DMA + pipeline granularity)
N_SUB = W // SUB  # 28 slices
RING_Z = 12  # input-slice ring depth (in-DMA prefetch window)
RING_M = 4   # mem-tile ring depth (consumed by ACT within ~1 slice)
RING_S = 12  # sign-tile ring depth (deep: out-DMAs queue behind the initial
RING_P = 12  # in-DMA burst on the FIFO DMA device, so sb/pb slots must not
#              be needed again until that burst drains)
# Output packing: for PACKED slices DVE packs the int8 sign pairs
# (odd*4 + even) so their out-DMA moves half the bytes. The count balances
# the DVE chain (+~1.0us per packed slice) against the serialized DMA
# device (-~0.32us per packed slice): the DVE-side critical path
# (fill + 65.8 + 0.994N + tail) crosses the DMA-side one
# (lead + 89.2 - 0.319N) near N=12 (swept empirically; Pool cannot help:
# its ISA rejects int8 adds). Spread evenly; the last slice stays
# unpacked to keep the drain tail short.
NPACK = 13
PACKED = tuple(
    s for s in range(N_SUB - 2)
    if (s * NPACK) // (N_SUB - 2) != ((s + 1) * NPACK) // (N_SUB - 2)
)
PIDX = {s: i for i, s in enumerate(PACKED)}
_Alu = mybir.AluOpType

_cache = {}
_lif_op = None


def _get_lif_op():
    """Register the fused LIF-step op in concourse's custom-DVE table.

    body: out = (Src0*C0 + Src1) - C1*(Src0 > C1)
          (Src0=mem_{t-1}, Src1=x_t, C0=beta, C1=T)
    """
    global _lif_op
    if _lif_op is not None:
        return _lif_op
    import concourse.dve_ops as dve_ops
    from concourse.dve_ops import DveOp
    from concourse.dve_spec import Spec, Src0, Src1, C0, C1, lower, _has_src1
    from concourse.dve_uop import DveOpSpec

    name = "LIF_STEP_ANT"
    for op in dve_ops.OPS:
        if op.name == name:
            _lif_op = op
            return op

    body = (Src0 * C0 + Src1) - C1 * (Src0 > C1)

    def ref(in0, in1, s0, s1, imm2):
        in0 = np.asarray(in0, np.float32)
        in1 = np.asarray(in1, np.float32)
        b = np.float32(s0) if not isinstance(s0, np.ndarray) else s0.astype(np.float32)
        t = np.float32(s1) if not isinstance(s1, np.ndarray) else s1.astype(np.float32)
        p = (in0 > t).astype(np.float32)
        return ((in0 * b + in1) - t * p).astype(np.float32)

    spec = Spec(body=body, reference=ref)
    row = max(dve_ops._SUB_OPCODE_FOR_NAME.values()) + 1
    assert row < 0x20
    op = DveOp(name, spec, subdim=False, uops_sha={})
    # sha is self-computed: pins the lowering within this process, the same
    # check dve_table_for_ops performs at compile time.
    for ver in ("v3", "v4"):
        s = DveOpSpec(
            name=name, opcode=row, uops=lower(spec, ver=ver), rd1_en=_has_src1(spec)
        )
        op.uops_sha[ver] = s.sha(ver)
    dve_ops._SUB_OPCODE_FOR_NAME[name] = row
    dve_ops.OPS.append(op)
    dve_ops.CUSTOM_DVE_SPECS[name] = spec
    _lif_op = op
    return op


def _lower_custom_dve(nc):
    """Encode InstCustomDveAnt instructions to ISA bytes in place.

    Raw Bass never runs Bacc's codegen_inst_isa_subclasses pass (and the
    blanket pass would also rewrite DMA copies, which this walrus rejects),
    so lower just the custom-DVE instructions.
    """
    for f in nc.m.functions:
        for bb in f.blocks:
            for i, inst in enumerate(bb.instructions):
                if isinstance(inst, bass_isa.InstCustomDveAnt):
                    lowered = mybir.codegen_inst_isa_one(inst, nc._state, nc.isa)
                    if isinstance(lowered, list):
                        assert len(lowered) == 1
                        lowered = lowered[0]
                    bb.instructions[i] = lowered


def _build(T: float):
    lif = _get_lif_op()
    nc = bass.Bass(trn_type="TRN2")
    x_d = nc.dram_tensor("x", [PB, W * H], mybir.dt.float32, kind="ExternalInput")
    r_d = nc.dram_tensor("r", [PB, W * H], mybir.dt.int8, kind="ExternalOutput")

    SL = SUB * H  # elements per slice (per partition)

    with (
        nc.sbuf_tensor("xb", [PB, RING_Z * SL], mybir.dt.float32) as xb,
        nc.sbuf_tensor("mb", [PB, RING_M * SL], mybir.dt.float32) as mb,
        nc.sbuf_tensor("sb", [PB, RING_S * SL], mybir.dt.int8) as sb,
        nc.sbuf_tensor("pb", [PB, RING_P * (SL // 2)], mybir.dt.int8) as pb,
        nc.sbuf_tensor("z0", [PB, H], mybir.dt.float32) as z0,
        nc.sbuf_tensor("nt", [PB, 1], mybir.dt.float32) as nt,
        nc.semaphore() as in_sem,
        nc.semaphore() as dl_sem,
        nc.semaphore() as v_sem,
        nc.semaphore() as p_sem,
        nc.semaphore() as act_sem,
        nc.semaphore() as out_sem,
        nc.Block() as block,
    ):

        @block.sync
        def _(sync):
            for s in range(N_SUB):
                if s >= RING_Z:
                    # x slot free once DVE consumed slice s-RING_Z
                    sync.wait_ge(v_sem, s - RING_Z + 1)
                sync.dma_start(
                    xb[:, (s % RING_Z) * SL : (s % RING_Z + 1) * SL],
                    x_d[:, s * SL : (s + 1) * SL],
                ).then_inc(in_sem, 16)
            # drain out-DMAs: SP is idle once the input stream is issued, and
            # its DMA-issue path (SEQ+HWDGE+DGE) overlaps ACT's sign pipeline
            # instead of queueing behind it.
            sd = N_SUB - 1
            HS2 = SL // 2
            sync.wait_ge(act_sem, N_SUB - 1)
            sync.dma_start(
                r_d[:, (sd - 1) * SL : sd * SL],
                sb[:, ((sd - 1) % RING_S) * SL : ((sd - 1) % RING_S + 1) * SL],
            ).then_inc(out_sem, 16)
            for h in range(2):
                if h == 0:
                    sync.wait_ge(act_sem, N_SUB)
                else:
                    sync.wait_ge(dl_sem, 1)
                sync.dma_start(
                    r_d[:, sd * SL + h * HS2 : sd * SL + (h + 1) * HS2],
                    sb[:, (sd % RING_S) * SL + h * HS2 : (sd % RING_S) * SL + (h + 1) * HS2],
                ).then_inc(out_sem, 16)

        @block.vector
        def _(vector):
            nc.vector.memset(z0[:], 0.0)
            nc.vector.memset(nt[:], -T)
            for s in range(N_SUB):
                vector.wait_ge(in_sem, 16 * (s + 1))
                if s >= RING_M:
                    # mem slot free once ACT signed slice s-RING_M
                    vector.wait_ge(act_sem, s - RING_M + 1)
                for j in range(SUB):
                    t = s * SUB + j
                    if t == 0:
                        src0 = z0[:]
                    else:
                        ps, pj = divmod(t - 1, SUB)
                        src0 = mb[
                            :, (ps % RING_M) * SL + pj * H : (ps % RING_M) * SL + (pj + 1) * H
                        ]
                    ins = nc.vector._custom_dve(
                        lif,
                        out=mb[:, (s % RING_M) * SL + j * H : (s % RING_M) * SL + (j + 1) * H],
                        in0=src0,
                        in1=xb[:, (s % RING_Z) * SL + j * H : (s % RING_Z) * SL + (j + 1) * H],
                        s0=BETA,
                        s1=T,
                    )
                    if s == N_SUB - 1:
                        # half-slice completion so the drain's sign/out-DMA
                        # can start on the first half early
                        if j == SUB // 2 - 1:
                            ins.then_inc(v_sem, 1)
                        elif j == SUB - 1:
                            ins.then_inc(v_sem, 1)
                            # second half's spikes computed here (527ns,
                            # program order) instead of ACT's 932ns sign
                            # behind a cross-engine wait
                            HS = SL // 2
                            sbase = (s % RING_S) * SL
                            mbase2 = (s % RING_M) * SL
                            nc.vector.tensor_scalar(
                                sb[:, sbase + HS : sbase + SL],
                                mb[:, mbase2 + HS : mbase2 + SL],
                                T,
                                1.0,
                                _Alu.is_gt,
                                _Alu.mult,
                            ).then_inc(dl_sem, 1)
                    elif j == SUB - 1:
                        ins.then_inc(v_sem, 1)
                # pack slice s-1 (one slice of lag hides ACT's sign latency)
                k = s - 1
                if k in PIDX:
                    kb = (k % RING_S) * SL
                    if k >= RING_P:
                        # pb slot free once out-DMA of slice k-RING_P completed
                        vector.wait_ge(out_sem, 16 * (k - RING_P + 1))
                    vector.wait_ge(act_sem, k + 1)
                    nc.vector.scalar_tensor_tensor(
                        pb[:, (k % RING_P) * (SL // 2) : (k % RING_P + 1) * (SL // 2)],
                        sb[:, kb + 1 : kb + SL : 2],
                        4.0,
                        sb[:, kb : kb + SL : 2],
                        _Alu.mult,
                        _Alu.add,
                    ).then_inc(p_sem, 1)

        @block.scalar
        def _(scalar):
            def emit_dma(k):
                # out-DMA for slice k, issued one slice late so its producer
                # (DVE's pack, or this engine's own sign) has already landed
                # by the time the SEQ reaches the wait.
                if k in PIDX:
                    scalar.wait_ge(p_sem, PIDX[k] + 1)
                    scalar.dma_start(
                        r_d[:, k * SL : k * SL + SL // 2],
                        pb[:, (k % RING_P) * (SL // 2) : (k % RING_P + 1) * (SL // 2)],
                    ).then_inc(out_sem, 16)
                else:
                    scalar.wait_ge(act_sem, k + 1)
                    scalar.dma_start(
                        r_d[:, k * SL : (k + 1) * SL],
                        sb[:, (k % RING_S) * SL : (k % RING_S + 1) * SL],
                    ).then_inc(out_sem, 16)

            for s in range(N_SUB - 1):
                if s >= RING_S:
                    # sb slot (slice s-RING_S) free once its consumer is done:
                    # DVE's pack for packed slices, the out-DMA otherwise
                    k = s - RING_S
                    if k in PIDX:
                        scalar.wait_ge(p_sem, PIDX[k] + 1)
                    else:
                        scalar.wait_ge(out_sem, 16 * (k + 1))
                base = (s % RING_S) * SL
                scalar.wait_ge(v_sem, s + 1)
                nc.scalar.sign(
                    sb[:, base : base + SL],
                    mb[:, (s % RING_M) * SL : (s % RING_M + 1) * SL],
                    bias=nt[:],
                ).then_inc(act_sem, 1)
                if s >= 1:
                    emit_dma(s - 1)
            # drain: slice 27 in two 4-step halves. v_sem counts 27 whole
            # slices then one per half (28, 29); act_sem likewise. Slice 26
            # is unpacked so its DMA (ready as soon as sign 26 landed) issues
            # before the halves; the final half-DMA needs no semaphore update
            # (nothing consumes it), saving the trailing propagation delay.
            s = N_SUB - 1
            k = s - RING_S
            if k in PIDX:
                scalar.wait_ge(p_sem, PIDX[k] + 1)
            else:
                scalar.wait_ge(out_sem, 16 * (k + 1))
            base = (s % RING_S) * SL
            mbase = (s % RING_M) * SL
            HS = SL // 2
            scalar.wait_ge(v_sem, s + 1)
            nc.scalar.sign(
                sb[:, base : base + HS],
                mb[:, mbase : mbase + HS],
                bias=nt[:],
            ).then_inc(act_sem, 1)

    _lower_custom_dve(nc)
    return nc


def kernel(inp: np.ndarray, t: np.ndarray, roll_amount) -> np.ndarray:
    T = float(
        np.clip(np.float32(np.asarray(t).reshape(-1)[0]), np.float32(1.0),
                np.float32(5.0))
    )
    roll = int(np.asarray(roll_amount)) % W

    key = (T,)
    if key not in _cache:
        _cache[key] = _build(T)
    nc = _cache[key]

    inp = np.asarray(inp, dtype=np.float32).reshape(B, CH, CH)
    in_maps = []
    for c in range(N_CORES):
        shard = inp[c * PB : (c + 1) * PB]  # (128, H, W)
        shard = np.roll(shard, roll, axis=2)
        # time-major: (128, W, H) contiguous
        x_tm = np.ascontiguousarray(shard.transpose(0, 2, 1)).reshape(PB, W * H)
        in_maps.append({"x": x_tm})

    res = run_bass_kernel_spmd(nc, in_maps, core_ids=list(range(N_CORES)))

    SL = SUB * H
    out = np.empty((B, 1, CH, CH), dtype=np.float32)
    for c in range(N_CORES):
        r = res.results[c]["r"]  # (PB, W*H) int8; packed slices use half a slot
        spk = np.empty((PB, W * H), dtype=bool)
        for s in range(N_SUB):
            if s in PIDX:
                v = r[:, s * SL : s * SL + SL // 2].astype(np.int32)
                odd = np.rint(v / 4.0).astype(np.int32)  # in {-1, 0, 1}
                even = v - 4 * odd
                sl = np.empty((PB, SL), dtype=bool)
                sl[:, 0::2] = even > 0
                sl[:, 1::2] = odd > 0
                spk[:, s * SL : (s + 1) * SL] = sl
            else:
                spk[:, s * SL : (s + 1) * SL] = r[:, s * SL : (s + 1) * SL] > 0
        spk = spk.reshape(PB, W, H)  # (b, w, h)
        out[c * PB : (c + 1) * PB, 0] = spk.transpose(0, 2, 1).astype(np.float32)
    return out
